# revision 62
# baseline (speedup 1.0000x reference)
# Trainium2 Bass kernel for nn_AttnBlock (GroupNorm + single-head NxN attention + proj + residual).
#
# Sharding: 8 cores = batch (4) x token-half (2). Each core receives its batch's
# x as (C=256, N=4096) with the token axis rolled so that the core's 2048 query
# tokens sit at local positions 0..2047. GroupNorm stats / k / v are
# token-permutation invariant, so every core computes GN and full k/v locally
# and attention rows only for its half — no collectives.
#
# Engine-balance design: softmax exp over 2048x4096 scores (8.4M elems) must be
# evacuated from PSUM, and only ACT and DVE have PSUM ports, so those two
# engines are the wall (~1 elem/cycle/lane each):
#   - exp tiles split across ACT (real exp -> fp8) and DVE (Schraudolph
#     bit-trick exp: fp8e4m3 bits = round(arg*8/ln2+B8), saturating uint8
#     convert via AP bitcast — one tensor_scalar per tile). All other PSUM
#     evacuations (kk/v/h2/out) are split A/V by sim-tuned patterns (PAT).
#   - the q-projection is eliminated: scores = h^T (wq^T wk) h with
#     M = wk^T wq precomputed host-side; kk = M h is the only projection.
#     Per-query bias cross-terms cancel in softmax. kk/v projections are
#     fp8 DoubleRow matmuls (2x PE rate).
#   - bo and the x residual are accumulated into the out-proj PSUM by ones/128
#     and identity matmuls; vbias likewise into the v PSUM — all evacuations
#     are plain copies/scales assignable to either ACT or DVE.
#   - x arrives as 6 merged slab DMAs sized so the bn_stats stream (serial
#     on DVE) starts as early as possible and ends ~0.7us after the last
#     byte; all bn_stats chunks are 512 tokens (bn_aggr combines variances
#     unweighted, so chunks MUST be equal-sized).
#   - the GN sqrt is followed by a dummy exp that consumes its output: the
#     ACT exp-table (re)load (1.28us) then executes during the still-idle
#     front window instead of at the start of the saturated exp phase
#     (x8 slab 0 converts on DVE so ACT's queue is empty there).
#   - HWDGE descriptor processing (~625ns/DMA) is a serialized shared
#     resource: constants queue AFTER x so the x stream runs at the DMA-bus
#     floor (~11.6us); outputs merge 2 blocks/DMA.
#   - PSUM: 8 banks = scores/kk/out slots (3x2-bank "sps") + v/h2/pair
#     accumulators (2x1-bank "acc"); score emission runs a half-chunk ahead
#     of h2 consumption.

import numpy as np

B, C, HH, WW = 4, 256, 64, 64
N = HH * WW           # 4096 tokens
NL = N // 2           # 2048 local query tokens per core
P = 128
EPS = 1e-5
NCORES = 8

_CACHE = {}

# Schraudolph fp8 exp constants: bits = round(arg * 8/ln2 + B8C), arg = s*scale - 4
A8 = 8.0 / np.log(2.0)
B8C = 56.5


def _mk_pattern(counts, n):
    out = []
    acc = {k: 0.0 for k in counts}
    for i in range(n):
        k = max(counts, key=lambda e: counts[e] / n * (i + 1) - acc[e])
        acc[k] += 1.0
        out.append(k)
    return out


# Feature flags (sim-bisection knobs)
F_X8_SPLIT = False    # x8 slabs 1-2 on ACT/DVE instead of Pool
F_KK_MERGE = False    # kk evac as one [P,2,512] op instead of 2x[P,512]
F_XDMA_2Q = False     # x DMA parts alternate SP/ACT queues
F_WARMUP = False      # PE pstate warmup matmuls
F_WPAIR_V = False     # wpair casts on DVE instead of Pool

# Per-instruction engine assignment counts (A, V); patterns built at
# _build_nc time so the sim search can override them.
PAT = {
    "exp_pro": (8, 8),     # 16 prologue exp tiles
    "exp_main": (30, 18),  # 48 main-phase exp tiles
    "kq": (7, 9),          # kk evacs (16 split / 8 merged)
    "ve": (11, 5),         # v evacs
    "oe": (2, 6),          # out evacs
    "h2": (0, 16),         # h2 scales
}


def _patterns():
    exp = _mk_pattern(dict(zip("AV", PAT["exp_pro"])), 16) + \
          _mk_pattern(dict(zip("AV", PAT["exp_main"])), 48)
    nkq = 8 if F_KK_MERGE else 16
    kq = _mk_pattern(dict(zip("AV", PAT["kq"])), nkq)
    ve = _mk_pattern(dict(zip("AV", PAT["ve"])), 16)
    oe = _mk_pattern(dict(zip("AV", PAT["oe"])), 8)
    h2 = _mk_pattern(dict(zip("AV", PAT["h2"])), 16)
    # the final half's two h2 scales run in parallel on A and V (tail path)
    h2[-2:] = ["A", "V"]
    # first kk chunk evacs on ACT: at that moment DVE is converting x8
    # slab-0 hb=1 while ACT just finished the exp-table load
    pass  # kq force disabled
    return exp, kq, ve, oe, h2


def _build_nc(reps=1):
    import concourse.bass as bass
    import concourse.tile as tile
    from concourse import bacc, mybir

    f32 = mybir.dt.float32
    f32r = mybir.dt.float32r
    fp8 = mybir.dt.float8e4
    u8 = mybir.dt.uint8
    Alu = mybir.AluOpType
    Act = mybir.ActivationFunctionType

    EXP_ENG, KQ_ENG, VE_ENG, OE_ENG, H2_ENG = _patterns()

    nc = bacc.Bacc("TRN2", target_bir_lowering=False, debug=False, num_devices=NCORES)

    x_d = nc.dram_tensor("x", [C, N], f32r, kind="ExternalInput")
    wqk_d = nc.dram_tensor("wqk", [C, C], f32r, kind="ExternalInput")
    wvt_d = nc.dram_tensor("wvt", [C, C], f32r, kind="ExternalInput")
    wot_d = nc.dram_tensor("wot", [C, C], f32r, kind="ExternalInput")
    bv_d = nc.dram_tensor("bv", [C], f32r, kind="ExternalInput")
    bo_d = nc.dram_tensor("bo", [C], f32r, kind="ExternalInput")
    gnw_d = nc.dram_tensor("gnw", [C], f32, kind="ExternalInput")
    gnb_d = nc.dram_tensor("gnb", [C], f32, kind="ExternalInput")
    pairm_d = nc.dram_tensor("pairm", [P, P], f32, kind="ExternalInput")  # 0.5-scaled
    ident_d = nc.dram_tensor("ident", [P, P], f32r, kind="ExternalInput")
    out_d = nc.dram_tensor("out", [C, NL], f32, kind="ExternalOutput")

    KT = N // P        # 32 key-token tiles
    SCH = 512          # scores free-dim chunk
    QCH = NL // SCH    # 4
    SLAB = 1024        # x DMA / projection slab
    NSLAB = N // SLAB  # 4

    scale = float(C) ** -0.5
    s1_schr = scale * A8
    s2_schr = B8C - 4.0 * A8

    with tile.TileContext(nc) as tc:
        from contextlib import ExitStack

        ENG = {"V": nc.vector, "P": nc.gpsimd}

        def copy8(eng, out_ap, in_ap):
            if eng == "A":
                nc.scalar.copy(out_ap, in_ap)
            else:
                ENG[eng].tensor_copy(out_ap, in_ap)

        with ExitStack() as ctx:
            consts = ctx.enter_context(tc.tile_pool(name="consts", bufs=1))
            big = ctx.enter_context(tc.tile_pool(name="big", bufs=1))
            small = ctx.enter_context(tc.tile_pool(name="small", bufs=1))
            etp = ctx.enter_context(tc.tile_pool(name="etp", bufs=36))
            outp = ctx.enter_context(tc.tile_pool(name="outp", bufs=4))
            psum = ctx.enter_context(tc.tile_pool(name="psum", bufs=1, space="PSUM"))

            loop_cm = tc.For_i(0, reps, 1) if reps > 1 else None
            if loop_cm is not None:
                ctx.enter_context(loop_cm)

            # ---------- x DMA: merged slab descriptors, then constants ----------
            # part sizes chosen so the bn_stats stream (serial on DVE, ~9.2us)
            # starts as early as possible and the LAST parts are small: the
            # final stats+aggregate tail after the last byte lands is ~0.7us.
            xh = big.tile([P, 2, N], f32r, name="xh")
            x8 = big.tile([P, 2, N], fp8, name="x8")
            # NOTE: bn_aggr combines chunk variances UNWEIGHTED, so all
            # bn_stats chunks must be the same size (512)
            xparts = [(0, 512), (512, 512), (1024, 1024), (2048, 1024),
                      (3072, 512), (3584, 512)]
            chunks = []
            for off, ln in xparts:
                o = off
                while o < off + ln:
                    c = min(512, off + ln - o)
                    chunks.append((o, c))
                    o += c
            st6 = [small.tile([P, len(chunks), 6], f32, name=f"st6_{ci}")
                   for ci in range(2)]
            ch_i = 0
            for pi, (off, ln) in enumerate(xparts):
                sl = slice(off, off + ln)
                nc.sync.dma_start(
                    xh[:, :, sl],
                    bass.AP(tensor=x_d, offset=off,
                            ap=[[N, P], [P * N, 2], [1, ln]]),
                )
                o = off
                while o < off + ln:
                    c = min(512, off + ln - o)
                    for ci in range(2):
                        nc.vector.bn_stats(out=st6[ci][:, ch_i, :],
                                           in_=xh[:, ci, o:o + c])
                    ch_i += 1
                    o += c

            # ---------- constants (merged descriptors, queued after x) ----------
            w2 = {}
            for wname, wd in (("kk", wqk_d), ("v", wvt_d), ("o", wot_d)):
                t = consts.tile([P, 2, C], f32r, name=f"w{wname}2_sb")
                nc.sync.dma_start(
                    t[:], bass.AP(tensor=wd, offset=0, ap=[[C, P], [P * C, 2], [1, C]])
                )
                w2[wname] = t
            w_sb = {(wn, ci): w2[wn][:, ci, :] for wn in ("kk", "v", "o") for ci in range(2)}

            pairm_sb = consts.tile([P, P], f32, name="pairm_sb")
            nc.sync.dma_start(pairm_sb[:], pairm_d.ap())
            ident_sb = consts.tile([P, P], f32r, name="ident_sb")
            nc.sync.dma_start(ident_sb[:], ident_d.ap())

            gnw2 = consts.tile([P, 2], f32, name="gnw2")
            nc.sync.dma_start(gnw2[:], bass.AP(tensor=gnw_d, offset=0, ap=[[1, P], [P, 2]]))
            gnb2 = consts.tile([P, 2], f32, name="gnb2")
            nc.sync.dma_start(gnb2[:], bass.AP(tensor=gnb_d, offset=0, ap=[[1, P], [P, 2]]))
            vbias_sb = consts.tile([P, C], f32r, name="vbias_sb")
            nc.sync.dma_start(
                vbias_sb[:], bass.AP(tensor=bv_d, offset=0, ap=[[0, P], [1, C]])
            )
            # bo as a broadcast row (f32r) for the PSUM-init matmul trick
            bo_rep = consts.tile([P, C], f32r, name="bo_rep")
            nc.sync.dma_start(
                bo_rep[:], bass.AP(tensor=bo_d, offset=0, ap=[[0, P], [1, C]])
            )
            ones_r = consts.tile([P, P], f32r, name="ones_r")
            nc.vector.memset(ones_r[:].bitcast(f32), 1.0 / P)

            eps_sb = consts.tile([P, 1], f32, name="eps_sb")
            nc.vector.memset(eps_sb[:], EPS)
            shift_sb = consts.tile([P, 1], f32, name="shift_sb")
            nc.vector.memset(shift_sb[:], -4.0)



            # ---------- GroupNorm coefficients a, b — vectorized over both halves ----------
            mv2 = small.tile([P, 2, 2], f32, name="mv2")
            for ci in range(2):
                nc.vector.bn_aggr(out=mv2[:, ci, :], in_=st6[ci][:])
            stats2 = small.tile([P, 2, 2], f32, name="stats2")  # (ci, [mean, E x^2])
            nc.vector.tensor_mul(stats2[:, :, 1], mv2[:, :, 0], mv2[:, :, 0])
            nc.vector.tensor_add(stats2[:, :, 1], stats2[:, :, 1], mv2[:, :, 1])
            nc.vector.tensor_copy(stats2[:, :, 0], mv2[:, :, 0])
            # pairm is 0.5-scaled -> per-pair [mean_g, E_g[x^2]] for both halves at once
            pair_ps = psum.tile([P, 4], f32, name="pair_ps", tag="acc", bufs=2)
            nc.tensor.matmul(pair_ps[:], pairm_sb[:], stats2[:], start=True, stop=True)
            pairs_t = small.tile([P, 2, 2], f32, name="pairs")
            nc.vector.tensor_copy(pairs_t[:], pair_ps[:])
            pairs = pairs_t[:]
            var_g = small.tile([P, 2], f32, name="var_g")
            nc.vector.tensor_mul(var_g[:], pairs[:, :, 0], pairs[:, :, 0])
            nc.vector.tensor_tensor(var_g[:], pairs[:, :, 1], var_g[:], Alu.subtract)
            sqv = small.tile([P, 2], f32, name="sqv")
            nc.scalar.activation(sqv[:], var_g[:], Act.Sqrt, bias=eps_sb[:], scale=1.0)
            # dummy exp consuming the sqrt output: pinned after the sqrt by
            # the data dependency, so the exp-table (re)load executes at
            # ~18us while ACT is otherwise idle (x8 slab 0 runs on DVE),
            # instead of costing 1.28us mid-exp-phase
            dummy = small.tile([P, 1], f32, name="dummy")
            nc.scalar.activation(dummy[:], sqv[:, 0:1], Act.Exp, scale=1.0)
            rstd = small.tile([P, 2], f32, name="rstd")
            nc.vector.reciprocal(rstd[:], sqv[:])
            a2 = small.tile([P, 2], f32, name="a2")
            nc.vector.tensor_mul(a2[:], rstd[:], gnw2[:])
            b2 = small.tile([P, 2], f32, name="b2")
            nc.vector.tensor_mul(b2[:], pairs[:, :, 0], a2[:])
            nc.vector.tensor_tensor(b2[:], gnb2[:], b2[:], Alu.subtract)
            ab = [(a2[:, ci:ci + 1], b2[:, ci:ci + 1]) for ci in range(2)]

            # ---------- fold GN affine into q/k/v weights (fp8 pair layout) ----------
            # casts on DVE: Pool is the prologue pacer, keep it free
            wpair = {}
            weng = nc.vector if F_WPAIR_V else nc.gpsimd
            for wname in ("kk", "v"):
                t = consts.tile([P, 2, C], fp8, name=f"w{wname}p_sb")
                for ci in range(2):
                    weng.tensor_copy(t[:, ci, :], w_sb[wname, ci])
                wpair[wname] = t

            # normalized h in fp8 pair layout: h = a*x + b, per channel.
            # Slab 0 gates the first projections. hb=0 (tokens 0-511, the
            # first kk chunk + first q-chunk) entirely on DVE so ACT's queue
            # stays free for the overlapped exp-table load; hb=1 split
            # ACT/DVE (the load is done by then).
            for ci in range(2):
                nc.vector.tensor_scalar(x8[:, ci, 0:SCH], xh[:, ci, 0:SCH],
                                        ab[ci][0], ab[ci][1],
                                        op0=Alu.mult, op1=Alu.add)
            nc.scalar.activation(x8[:, 0, SCH:2 * SCH], xh[:, 0, SCH:2 * SCH],
                                 Act.Identity, bias=ab[0][1], scale=ab[0][0])
            nc.vector.tensor_scalar(x8[:, 1, SCH:2 * SCH], xh[:, 1, SCH:2 * SCH],
                                    ab[1][0], ab[1][1],
                                    op0=Alu.mult, op1=Alu.add)
            for s in range(1, NSLAB):
                sl = slice(s * SLAB, (s + 1) * SLAB)
                if F_X8_SPLIT and s < 3:
                    nc.scalar.activation(x8[:, 0, sl], xh[:, 0, sl], Act.Identity,
                                         bias=ab[0][1], scale=ab[0][0])
                    nc.vector.tensor_scalar(x8[:, 1, sl], xh[:, 1, sl],
                                            ab[1][0], ab[1][1],
                                            op0=Alu.mult, op1=Alu.add)
                else:
                    for ci in range(2):
                        nc.gpsimd.tensor_scalar(x8[:, ci, sl], xh[:, ci, sl],
                                                ab[ci][0], ab[ci][1],
                                                op0=Alu.mult, op1=Alu.add)

            # ---------- PE p-state warmup ----------
            # keep the PE busy through the tail of the GN-stats window so the
            # 3us pstate ramp completes before the first real matmul burst
            if F_WARMUP:
                for wi in range(8):
                    wps = psum.tile([P, SCH], f32, name=f"warm_{wi}", tag="sps", bufs=3)
                    nc.tensor.matmul(wps[:], ones_r[:],
                                     xh[:, 0, 3 * SLAB + SCH:3 * SLAB + 2 * SCH],
                                     start=True, stop=True)

            # ---------- kk = (wk^T wq applied) projection (fp8 pair layout) ----------
            kT_pair = big.tile([P, 2, N], fp8, name="kT_pair")

            kq_i = [0]

            def emit_kkproj(s):
                # one 512-token chunk, both output-channel halves
                sl = slice(s * SCH, (s + 1) * SCH)
                if F_KK_MERGE:
                    # one 2-bank psum tile -> single merged evac op
                    ps = psum.tile([P, 2, SCH], f32, name=f"kkps_{s}",
                                   tag="sps", bufs=3)
                    for co in range(2):
                        nc.tensor.matmul(ps[:, co, :],
                                         wpair["kk"][:, :, co * P:(co + 1) * P],
                                         x8[:, :, sl], start=True, stop=True,
                                         perf_mode=mybir.MatmulPerfMode.DoubleRow)
                    eng = KQ_ENG[kq_i[0] % len(KQ_ENG)]
                    kq_i[0] += 1
                    copy8(eng, kT_pair[:, :, sl], ps[:])
                else:
                    for co in range(2):
                        ps = psum.tile([P, SCH], f32, name=f"kkps_{co}_{s}",
                                       tag="sps", bufs=3)
                        nc.tensor.matmul(ps[:], wpair["kk"][:, :, co * P:(co + 1) * P],
                                         x8[:, :, sl], start=True, stop=True,
                                         perf_mode=mybir.MatmulPerfMode.DoubleRow)
                        copy8(KQ_ENG[kq_i[0] % len(KQ_ENG)], kT_pair[:, co, sl], ps[:])
                        kq_i[0] += 1

            # v in (token on partitions, channel free) fp8 pair layout with ones col
            CP = 272  # C+1 padded to a 16B multiple for the DoubleRow ko-stride
            v_sb = big.tile([P, KT // 2, 2, CP], fp8, name="v_sb")
            nc.vector.memset(v_sb[:, :, :, C:], 0.0)
            nc.vector.memset(v_sb[:, :, :, C:C + 1], 1.0)

            ve_i = [0]

            def emit_v(ktp):
                ps = psum.tile([P, 2, C], f32, name=f"vps_{ktp}", tag="acc", bufs=2)
                for j in range(2):
                    kt = 2 * ktp + j
                    tsl = slice(kt * P, (kt + 1) * P)
                    nc.tensor.matmul(ps[:, j, :], ones_r[:], vbias_sb[:],
                                     start=True, stop=False)
                    nc.tensor.matmul(ps[:, j, :], x8[:, :, tsl], wpair["v"][:],
                                     start=False, stop=True,
                                     perf_mode=mybir.MatmulPerfMode.DoubleRow)
                eng = VE_ENG[ve_i[0] % len(VE_ENG)]
                ve_i[0] += 1
                copy8(eng, v_sb[:, ktp, :, 0:C], ps[:])

            # ---------- attention, software-pipelined ----------
            et_chunks = [[None] * (KT // 2) for _ in range(QCH)]

            def emit_score_pair(qc, ktp):
                qsl = slice(qc * SCH, (qc + 1) * SCH)
                ets = et_chunks[qc]
                ets[ktp] = etp.tile([P, 2, SCH], fp8, name=f"et_{qc}_{ktp}", tag="et")
                ps2 = psum.tile([P, 2, SCH], f32, name=f"sps_{qc}_{ktp}", tag="sps", bufs=3)
                for j in range(2):
                    kt = 2 * ktp + j
                    nc.tensor.matmul(ps2[:, j, :], kT_pair[:, :, kt * P:(kt + 1) * P],
                                     x8[:, :, qsl], start=True, stop=True,
                                     perf_mode=mybir.MatmulPerfMode.DoubleRow)
                eng = EXP_ENG[(qc * (KT // 2) + ktp) % len(EXP_ENG)]
                if eng == "A":
                    nc.scalar.activation(ets[ktp][:], ps2[:], Act.Exp,
                                         scale=scale, bias=shift_sb[:])
                else:
                    ENG[eng].tensor_scalar(ets[ktp][:].bitcast(u8), ps2[:],
                                           s1_schr, s2_schr,
                                           op0=Alu.mult, op1=Alu.add)

            # prologue: q slab 0, then per 1024-token slab: k-proj followed by
            # its 4 score pairs for q-chunk 0, v tiles, remaining q slab
            for sc in range(2 * NSLAB):
                emit_kkproj(sc)
                for ktp in (2 * sc, 2 * sc + 1):
                    emit_score_pair(0, ktp)
                    emit_v(ktp)

            oe_i = [0]

            def emit_final(rr):
                osb = outp.tile([P, 2, C], f32, name=f"osb_{rr}", tag="osb", bufs=3)
                ps = psum.tile([P, 2, C], f32, name=f"ops_{rr}", tag="sps", bufs=3)
                for mt in range(2):
                    msl = slice(mt * P, (mt + 1) * P)
                    nc.tensor.matmul(ps[:, mt, :], ones_r[:], bo_rep[:],
                                     start=True, stop=False)
                    nc.tensor.matmul(ps[:, mt, :], ident_sb[:],
                                     xh[:, mt, rr * C:(rr + 1) * C],
                                     start=False, stop=False)
                    nc.tensor.matmul(ps[:, mt, :], h2[2 * rr][:, msl], w_sb["o", 0],
                                     start=False, stop=False)
                    nc.tensor.matmul(ps[:, mt, :], h2[2 * rr + 1][:, msl], w_sb["o", 1],
                                     start=False, stop=True)
                eng = OE_ENG[oe_i[0] % len(OE_ENG)]
                oe_i[0] += 1
                copy8(eng, osb[:], ps[:])
                # one merged DMA for both 128-row blocks of this 256-token column set
                nc.sync.dma_start(
                    bass.AP(tensor=out_d, offset=rr * C,
                            ap=[[NL, P], [P * NL, 2], [1, C]]),
                    osb[:],
                )

            h2 = []
            h2_i = [0]
            for qc in range(QCH):
                ets = et_chunks[qc]
                for half in range(2):
                    hpss = [
                        psum.tile([P, CP], f32, name=f"hps_{qc}_{half}_{j}",
                                  tag="acc", bufs=2)
                        for j in range(2)
                    ]
                    for ktp in range(KT // 2):
                        for j in range(2):
                            qt = 2 * half + j
                            nc.tensor.matmul(hpss[j][:],
                                             ets[ktp][:, :, qt * P:(qt + 1) * P],
                                             v_sb[:, ktp, :, :],
                                             start=(ktp == 0), stop=(ktp == KT // 2 - 1),
                                             perf_mode=mybir.MatmulPerfMode.DoubleRow)
                        g = half * (KT // 2) + ktp
                        if qc + 1 < QCH and g % 2 == 0:
                            emit_score_pair(qc + 1, g // 2)
                    for j in range(2):
                        qt = 2 * half + j
                        rec = small.tile([P, 1], f32, name=f"rec_{qc}_{qt}", tag="rec", bufs=4)
                        nc.vector.reciprocal(rec[:], hpss[j][:, C:C + 1])
                        h2t = big.tile([P, C], f32r, name=f"h2_{qc}_{qt}", tag="h2", bufs=6)
                        eng = H2_ENG[h2_i[0] % len(H2_ENG)]
                        h2_i[0] += 1
                        if eng == "A":
                            nc.scalar.mul(h2t[:], hpss[j][:, 0:C], rec[:])
                        else:
                            ENG[eng].tensor_scalar_mul(h2t[:], hpss[j][:, 0:C], rec[:])
                        h2.append(h2t)
                    # final projection for the 256-token block this half completed
                    emit_final(2 * qc + half)

    nc.compile()
    return nc


def _get_nc():
    if "nc" not in _CACHE:
        _CACHE["nc"] = _build_nc()
    return _CACHE["nc"]


def _make_in_maps(x, gn_w, gn_b, wq, bq, wk, bk, wv, bv, wo, bo):
    x = np.ascontiguousarray(np.asarray(x, dtype=np.float32)).reshape(B, C, N)
    pairm = np.zeros((P, P), dtype=np.float32)
    idx = np.arange(P)
    pairm[idx[:, None] // 2 == idx[None, :] // 2] = 0.5
    wqf = np.asarray(wq, np.float64)
    wkf = np.asarray(wk, np.float64)
    common = {
        "wqk": np.ascontiguousarray((wkf.T @ wqf).astype(np.float32)),
        "wvt": np.ascontiguousarray(np.asarray(wv, np.float32).T),
        "wot": np.ascontiguousarray(np.asarray(wo, np.float32).T),
        "bv": np.asarray(bv, np.float32),
        "bo": np.asarray(bo, np.float32),
        "gnw": np.asarray(gn_w, np.float32),
        "gnb": np.asarray(gn_b, np.float32),
        "pairm": pairm,
        "ident": np.eye(P, dtype=np.float32),
    }
    in_maps = []
    for core in range(NCORES):
        b, half = divmod(core, 2)
        xs = np.roll(x[b], -NL * half, axis=1) if half else x[b]
        in_maps.append({**common, "x": np.ascontiguousarray(xs)})
    return in_maps


def kernel(x, gn_w, gn_b, wq, bq, wk, bk, wv, bv, wo, bo):
    from concourse.bass_utils import run_bass_kernel_spmd

    nc = _get_nc()
    in_maps = _make_in_maps(x, gn_w, gn_b, wq, bq, wk, bk, wv, bv, wo, bo)
    res = run_bass_kernel_spmd(nc, in_maps, core_ids=list(range(NCORES)))
    _CACHE["last_result"] = res

    out = np.empty((B, C, N), dtype=np.float32)
    for core in range(NCORES):
        b, half = divmod(core, 2)
        out[b][:, NL * half:NL * (half + 1)] = res.results[core]["out"]
    return out.reshape(B, C, HH, WW)


# revision 69
# speedup vs baseline: 1.6454x; 1.6454x over previous
# Trainium2 Bass kernel for nn_AttnBlock (GroupNorm + single-head NxN attention + proj + residual).
#
# Sharding: 8 cores = batch (4) x token-half (2). Each core receives its batch's
# x as (C=256, N=4096) with the token axis rolled so that the core's 2048 query
# tokens sit at local positions 0..2047. GroupNorm stats / k / v are
# token-permutation invariant, so every core computes GN and full k/v locally
# and attention rows only for its half — no collectives.
#
# Engine-balance design: softmax exp over 2048x4096 scores (8.4M elems) must be
# evacuated from PSUM, and only ACT and DVE have PSUM ports, so those two
# engines are the wall (~1 elem/cycle/lane each):
#   - exp tiles split across ACT (real exp -> fp8) and DVE (Schraudolph
#     bit-trick exp: fp8e4m3 bits = round(arg*8/ln2+B8), saturating uint8
#     convert via AP bitcast — one tensor_scalar per tile). All other PSUM
#     evacuations (kk/v/h2/out) are split A/V by sim-tuned patterns (PAT).
#   - the q-projection is eliminated: scores = h^T (wq^T wk) h with
#     M = wk^T wq precomputed host-side; kk = M h is the only projection.
#     Per-query bias cross-terms cancel in softmax. kk/v projections are
#     fp8 DoubleRow matmuls (2x PE rate).
#   - bo and the x residual are accumulated into the out-proj PSUM by ones/128
#     and identity matmuls; vbias likewise into the v PSUM — all evacuations
#     are plain copies/scales assignable to either ACT or DVE.
#   - x arrives as 6 merged slab DMAs sized so the bn_stats stream (serial
#     on DVE) starts as early as possible and ends ~0.7us after the last
#     byte; all bn_stats chunks are 512 tokens (bn_aggr combines variances
#     unweighted, so chunks MUST be equal-sized).
#   - the GN sqrt is followed by a dummy exp that consumes its output: the
#     ACT exp-table (re)load (1.28us) then executes during the still-idle
#     front window instead of at the start of the saturated exp phase
#     (x8 slab 0 converts on DVE so ACT's queue is empty there).
#   - HWDGE descriptor processing (~625ns/DMA) is a serialized shared
#     resource: constants queue AFTER x so the x stream runs at the DMA-bus
#     floor (~11.6us); outputs merge 2 blocks/DMA.
#   - PSUM: 8 banks = scores/kk/out slots (3x2-bank "sps") + v/h2/pair
#     accumulators (2x1-bank "acc"); score emission runs a half-chunk ahead
#     of h2 consumption.

import numpy as np

B, C, HH, WW = 4, 256, 64, 64
N = HH * WW           # 4096 tokens
NL = N // 2           # 2048 local query tokens per core
P = 128
EPS = 1e-5
NCORES = 8

_CACHE = {}

# Schraudolph fp8 exp constants: bits = round(arg * 8/ln2 + B8C), arg = s*scale - 4
A8 = 8.0 / np.log(2.0)
B8C = 56.5


def _mk_pattern(counts, n):
    out = []
    acc = {k: 0.0 for k in counts}
    for i in range(n):
        k = max(counts, key=lambda e: counts[e] / n * (i + 1) - acc[e])
        acc[k] += 1.0
        out.append(k)
    return out


# Feature flags (sim-bisection knobs)
F_X8_SPLIT = False    # x8 slabs 1-2 on ACT/DVE instead of Pool
F_KK_MERGE = False    # kk evac as one [P,2,512] op instead of 2x[P,512]
F_XDMA_2Q = False     # x DMA parts alternate SP/ACT queues
F_WARMUP = False      # PE pstate warmup matmuls
F_WPAIR_V = False     # wpair casts on DVE instead of Pool

# Per-instruction engine assignment counts (A, V); patterns built at
# _build_nc time so the sim search can override them.
PAT = {
    "exp_pro": (8, 8),     # 16 prologue exp tiles
    "exp_q1": (10, 6),     # 16 exp tiles emitted during qc0's h2 loop
    "exp_q2": (10, 6),     # ... qc1's
    "exp_q3": (10, 6),     # ... qc2's
    "kq": (7, 9),          # kk evacs (16 split / 8 merged)
    "ve": (11, 5),         # v evacs
    "oe": (2, 6),          # out evacs
    "h2": (0, 16),         # h2 scales
}


def _patterns():
    exp = _mk_pattern(dict(zip("AV", PAT["exp_pro"])), 16)
    for k in ("exp_q1", "exp_q2", "exp_q3"):
        exp += _mk_pattern(dict(zip("AV", PAT[k])), 16)
    nkq = 8 if F_KK_MERGE else 16
    kq = _mk_pattern(dict(zip("AV", PAT["kq"])), nkq)
    ve = _mk_pattern(dict(zip("AV", PAT["ve"])), 16)
    oe = _mk_pattern(dict(zip("AV", PAT["oe"])), 8)
    h2 = _mk_pattern(dict(zip("AV", PAT["h2"])), 16)
    # the final half's two h2 scales run in parallel on A and V (tail path)
    h2[-2:] = ["A", "V"]
    # first kk chunk evacs on ACT: at that moment DVE is converting x8
    # slab-0 hb=1 while ACT just finished the exp-table load
    pass  # kq force disabled
    return exp, kq, ve, oe, h2


def _build_nc(reps=1):
    import concourse.bass as bass
    import concourse.tile as tile
    from concourse import bacc, mybir

    f32 = mybir.dt.float32
    f32r = mybir.dt.float32r
    fp8 = mybir.dt.float8e4
    u8 = mybir.dt.uint8
    Alu = mybir.AluOpType
    Act = mybir.ActivationFunctionType

    EXP_ENG, KQ_ENG, VE_ENG, OE_ENG, H2_ENG = _patterns()

    nc = bacc.Bacc("TRN2", target_bir_lowering=False, debug=False, num_devices=NCORES)

    x_d = nc.dram_tensor("x", [C, N], f32r, kind="ExternalInput")
    wqk_d = nc.dram_tensor("wqk", [C, C], f32r, kind="ExternalInput")
    wvt_d = nc.dram_tensor("wvt", [C, C], f32r, kind="ExternalInput")
    wot_d = nc.dram_tensor("wot", [C, C], f32r, kind="ExternalInput")
    bv_d = nc.dram_tensor("bv", [C], f32r, kind="ExternalInput")
    bo_d = nc.dram_tensor("bo", [C], f32r, kind="ExternalInput")
    gnw_d = nc.dram_tensor("gnw", [C], f32, kind="ExternalInput")
    gnb_d = nc.dram_tensor("gnb", [C], f32, kind="ExternalInput")
    pairm_d = nc.dram_tensor("pairm", [P, P], f32, kind="ExternalInput")  # 0.5-scaled
    ident_d = nc.dram_tensor("ident", [P, P], f32r, kind="ExternalInput")
    out_d = nc.dram_tensor("out", [C, NL], f32, kind="ExternalOutput")

    KT = N // P        # 32 key-token tiles
    SCH = 512          # scores free-dim chunk
    QCH = NL // SCH    # 4
    SLAB = 1024        # x DMA / projection slab
    NSLAB = N // SLAB  # 4

    scale = float(C) ** -0.5
    s1_schr = scale * A8
    s2_schr = B8C - 4.0 * A8

    with tile.TileContext(nc) as tc:
        from contextlib import ExitStack

        ENG = {"V": nc.vector, "P": nc.gpsimd}

        def copy8(eng, out_ap, in_ap):
            if eng == "A":
                nc.scalar.copy(out_ap, in_ap)
            else:
                ENG[eng].tensor_copy(out_ap, in_ap)

        with ExitStack() as ctx:
            consts = ctx.enter_context(tc.tile_pool(name="consts", bufs=1))
            big = ctx.enter_context(tc.tile_pool(name="big", bufs=1))
            small = ctx.enter_context(tc.tile_pool(name="small", bufs=1))
            etp = ctx.enter_context(tc.tile_pool(name="etp", bufs=36))
            outp = ctx.enter_context(tc.tile_pool(name="outp", bufs=4))
            psum = ctx.enter_context(tc.tile_pool(name="psum", bufs=1, space="PSUM"))

            loop_cm = tc.For_i(0, reps, 1) if reps > 1 else None
            if loop_cm is not None:
                ctx.enter_context(loop_cm)

            # ---------- x DMA: merged slab descriptors, then constants ----------
            # part sizes chosen so the bn_stats stream (serial on DVE, ~9.2us)
            # starts as early as possible and the LAST parts are small: the
            # final stats+aggregate tail after the last byte lands is ~0.7us.
            xh = big.tile([P, 2, N], f32r, name="xh")
            x8 = big.tile([P, 2, N], fp8, name="x8")
            # NOTE: bn_aggr combines chunk variances UNWEIGHTED, so all
            # bn_stats chunks must be the same size (512)
            xparts = [(0, 512), (512, 512), (1024, 1024), (2048, 1024),
                      (3072, 512), (3584, 512)]
            chunks = []
            for off, ln in xparts:
                o = off
                while o < off + ln:
                    c = min(512, off + ln - o)
                    chunks.append((o, c))
                    o += c
            st6 = [small.tile([P, len(chunks), 6], f32, name=f"st6_{ci}")
                   for ci in range(2)]
            ch_i = 0
            for pi, (off, ln) in enumerate(xparts):
                sl = slice(off, off + ln)
                nc.sync.dma_start(
                    xh[:, :, sl],
                    bass.AP(tensor=x_d, offset=off,
                            ap=[[N, P], [P * N, 2], [1, ln]]),
                )
                o = off
                while o < off + ln:
                    c = min(512, off + ln - o)
                    for ci in range(2):
                        nc.vector.bn_stats(out=st6[ci][:, ch_i, :],
                                           in_=xh[:, ci, o:o + c])
                    ch_i += 1
                    o += c

            # ---------- constants (merged descriptors, queued after x) ----------
            w2 = {}
            for wname, wd in (("kk", wqk_d), ("v", wvt_d), ("o", wot_d)):
                t = consts.tile([P, 2, C], f32r, name=f"w{wname}2_sb")
                nc.sync.dma_start(
                    t[:], bass.AP(tensor=wd, offset=0, ap=[[C, P], [P * C, 2], [1, C]])
                )
                w2[wname] = t
            w_sb = {(wn, ci): w2[wn][:, ci, :] for wn in ("kk", "v", "o") for ci in range(2)}

            pairm_sb = consts.tile([P, P], f32, name="pairm_sb")
            nc.sync.dma_start(pairm_sb[:], pairm_d.ap())
            ident_sb = consts.tile([P, P], f32r, name="ident_sb")
            nc.sync.dma_start(ident_sb[:], ident_d.ap())

            gnw2 = consts.tile([P, 2], f32, name="gnw2")
            nc.sync.dma_start(gnw2[:], bass.AP(tensor=gnw_d, offset=0, ap=[[1, P], [P, 2]]))
            gnb2 = consts.tile([P, 2], f32, name="gnb2")
            nc.sync.dma_start(gnb2[:], bass.AP(tensor=gnb_d, offset=0, ap=[[1, P], [P, 2]]))
            vbias_sb = consts.tile([P, C], f32r, name="vbias_sb")
            nc.sync.dma_start(
                vbias_sb[:], bass.AP(tensor=bv_d, offset=0, ap=[[0, P], [1, C]])
            )
            # bo as a broadcast row (f32r) for the PSUM-init matmul trick
            bo_rep = consts.tile([P, C], f32r, name="bo_rep")
            nc.sync.dma_start(
                bo_rep[:], bass.AP(tensor=bo_d, offset=0, ap=[[0, P], [1, C]])
            )
            ones_r = consts.tile([P, P], f32r, name="ones_r")
            nc.vector.memset(ones_r[:].bitcast(f32), 1.0 / P)

            eps_sb = consts.tile([P, 1], f32, name="eps_sb")
            nc.vector.memset(eps_sb[:], EPS)
            shift_sb = consts.tile([P, 1], f32, name="shift_sb")
            nc.vector.memset(shift_sb[:], -4.0)



            # ---------- GroupNorm coefficients a, b — vectorized over both halves ----------
            mv2 = small.tile([P, 2, 2], f32, name="mv2")
            for ci in range(2):
                nc.vector.bn_aggr(out=mv2[:, ci, :], in_=st6[ci][:])
            stats2 = small.tile([P, 2, 2], f32, name="stats2")  # (ci, [mean, E x^2])
            nc.vector.tensor_mul(stats2[:, :, 1], mv2[:, :, 0], mv2[:, :, 0])
            nc.vector.tensor_add(stats2[:, :, 1], stats2[:, :, 1], mv2[:, :, 1])
            nc.vector.tensor_copy(stats2[:, :, 0], mv2[:, :, 0])
            # pairm is 0.5-scaled -> per-pair [mean_g, E_g[x^2]] for both halves at once
            pair_ps = psum.tile([P, 4], f32, name="pair_ps", tag="acc", bufs=2)
            nc.tensor.matmul(pair_ps[:], pairm_sb[:], stats2[:], start=True, stop=True)
            pairs_t = small.tile([P, 2, 2], f32, name="pairs")
            nc.vector.tensor_copy(pairs_t[:], pair_ps[:])
            pairs = pairs_t[:]
            var_g = small.tile([P, 2], f32, name="var_g")
            nc.vector.tensor_mul(var_g[:], pairs[:, :, 0], pairs[:, :, 0])
            nc.vector.tensor_tensor(var_g[:], pairs[:, :, 1], var_g[:], Alu.subtract)
            sqv = small.tile([P, 2], f32, name="sqv")
            nc.scalar.activation(sqv[:], var_g[:], Act.Sqrt, bias=eps_sb[:], scale=1.0)
            # dummy exp consuming the sqrt output: pinned after the sqrt by
            # the data dependency, so the exp-table (re)load executes at
            # ~18us while ACT is otherwise idle (x8 slab 0 runs on DVE),
            # instead of costing 1.28us mid-exp-phase
            dummy = small.tile([P, 1], f32, name="dummy")
            nc.scalar.activation(dummy[:], sqv[:, 0:1], Act.Exp, scale=1.0)
            rstd = small.tile([P, 2], f32, name="rstd")
            nc.vector.reciprocal(rstd[:], sqv[:])
            a2 = small.tile([P, 2], f32, name="a2")
            nc.vector.tensor_mul(a2[:], rstd[:], gnw2[:])
            b2 = small.tile([P, 2], f32, name="b2")
            nc.vector.tensor_mul(b2[:], pairs[:, :, 0], a2[:])
            nc.vector.tensor_tensor(b2[:], gnb2[:], b2[:], Alu.subtract)
            ab = [(a2[:, ci:ci + 1], b2[:, ci:ci + 1]) for ci in range(2)]

            # ---------- fold GN affine into q/k/v weights (fp8 pair layout) ----------
            # casts on DVE: Pool is the prologue pacer, keep it free
            wpair = {}
            weng = nc.vector if F_WPAIR_V else nc.gpsimd
            for wname in ("kk", "v"):
                t = consts.tile([P, 2, C], fp8, name=f"w{wname}p_sb")
                for ci in range(2):
                    weng.tensor_copy(t[:, ci, :], w_sb[wname, ci])
                wpair[wname] = t

            # normalized h in fp8 pair layout: h = a*x + b, per channel.
            # Slab 0 gates the first projections. hb=0 (tokens 0-511, the
            # first kk chunk + first q-chunk) entirely on DVE so ACT's queue
            # stays free for the overlapped exp-table load; hb=1 split
            # ACT/DVE (the load is done by then).
            for ci in range(2):
                nc.vector.tensor_scalar(x8[:, ci, 0:SCH], xh[:, ci, 0:SCH],
                                        ab[ci][0], ab[ci][1],
                                        op0=Alu.mult, op1=Alu.add)
            nc.scalar.activation(x8[:, 0, SCH:2 * SCH], xh[:, 0, SCH:2 * SCH],
                                 Act.Identity, bias=ab[0][1], scale=ab[0][0])
            nc.vector.tensor_scalar(x8[:, 1, SCH:2 * SCH], xh[:, 1, SCH:2 * SCH],
                                    ab[1][0], ab[1][1],
                                    op0=Alu.mult, op1=Alu.add)
            for s in range(1, NSLAB):
                sl = slice(s * SLAB, (s + 1) * SLAB)
                if F_X8_SPLIT and s < 3:
                    nc.scalar.activation(x8[:, 0, sl], xh[:, 0, sl], Act.Identity,
                                         bias=ab[0][1], scale=ab[0][0])
                    nc.vector.tensor_scalar(x8[:, 1, sl], xh[:, 1, sl],
                                            ab[1][0], ab[1][1],
                                            op0=Alu.mult, op1=Alu.add)
                else:
                    for ci in range(2):
                        nc.gpsimd.tensor_scalar(x8[:, ci, sl], xh[:, ci, sl],
                                                ab[ci][0], ab[ci][1],
                                                op0=Alu.mult, op1=Alu.add)

            # ---------- PE p-state warmup ----------
            # keep the PE busy through the tail of the GN-stats window so the
            # 3us pstate ramp completes before the first real matmul burst
            if F_WARMUP:
                for wi in range(8):
                    wps = psum.tile([P, SCH], f32, name=f"warm_{wi}", tag="sps", bufs=3)
                    nc.tensor.matmul(wps[:], ones_r[:],
                                     xh[:, 0, 3 * SLAB + SCH:3 * SLAB + 2 * SCH],
                                     start=True, stop=True)

            # ---------- kk = (wk^T wq applied) projection (fp8 pair layout) ----------
            kT_pair = big.tile([P, 2, N], fp8, name="kT_pair")

            kq_i = [0]

            def emit_kkproj(s):
                # one 512-token chunk, both output-channel halves
                sl = slice(s * SCH, (s + 1) * SCH)
                if F_KK_MERGE:
                    # one 2-bank psum tile -> single merged evac op
                    ps = psum.tile([P, 2, SCH], f32, name=f"kkps_{s}",
                                   tag="sps", bufs=3)
                    for co in range(2):
                        nc.tensor.matmul(ps[:, co, :],
                                         wpair["kk"][:, :, co * P:(co + 1) * P],
                                         x8[:, :, sl], start=True, stop=True,
                                         perf_mode=mybir.MatmulPerfMode.DoubleRow)
                    eng = KQ_ENG[kq_i[0] % len(KQ_ENG)]
                    kq_i[0] += 1
                    copy8(eng, kT_pair[:, :, sl], ps[:])
                else:
                    for co in range(2):
                        ps = psum.tile([P, SCH], f32, name=f"kkps_{co}_{s}",
                                       tag="sps", bufs=3)
                        nc.tensor.matmul(ps[:], wpair["kk"][:, :, co * P:(co + 1) * P],
                                         x8[:, :, sl], start=True, stop=True,
                                         perf_mode=mybir.MatmulPerfMode.DoubleRow)
                        copy8(KQ_ENG[kq_i[0] % len(KQ_ENG)], kT_pair[:, co, sl], ps[:])
                        kq_i[0] += 1

            # v in (token on partitions, channel free) fp8 pair layout with ones col
            CP = 272  # C+1 padded to a 16B multiple for the DoubleRow ko-stride
            v_sb = big.tile([P, KT // 2, 2, CP], fp8, name="v_sb")
            nc.vector.memset(v_sb[:, :, :, C:], 0.0)
            nc.vector.memset(v_sb[:, :, :, C:C + 1], 1.0)

            ve_i = [0]

            def emit_v(ktp):
                ps = psum.tile([P, 2, C], f32, name=f"vps_{ktp}", tag="acc", bufs=2)
                for j in range(2):
                    kt = 2 * ktp + j
                    tsl = slice(kt * P, (kt + 1) * P)
                    nc.tensor.matmul(ps[:, j, :], ones_r[:], vbias_sb[:],
                                     start=True, stop=False)
                    nc.tensor.matmul(ps[:, j, :], x8[:, :, tsl], wpair["v"][:],
                                     start=False, stop=True,
                                     perf_mode=mybir.MatmulPerfMode.DoubleRow)
                eng = VE_ENG[ve_i[0] % len(VE_ENG)]
                ve_i[0] += 1
                copy8(eng, v_sb[:, ktp, :, 0:C], ps[:])

            # ---------- attention, software-pipelined ----------
            et_chunks = [[None] * (KT // 2) for _ in range(QCH)]

            def emit_score_pair(qc, ktp):
                qsl = slice(qc * SCH, (qc + 1) * SCH)
                ets = et_chunks[qc]
                ets[ktp] = etp.tile([P, 2, SCH], fp8, name=f"et_{qc}_{ktp}", tag="et")
                ps2 = psum.tile([P, 2, SCH], f32, name=f"sps_{qc}_{ktp}", tag="sps", bufs=3)
                for j in range(2):
                    kt = 2 * ktp + j
                    nc.tensor.matmul(ps2[:, j, :], kT_pair[:, :, kt * P:(kt + 1) * P],
                                     x8[:, :, qsl], start=True, stop=True,
                                     perf_mode=mybir.MatmulPerfMode.DoubleRow)
                eng = EXP_ENG[(qc * (KT // 2) + ktp) % len(EXP_ENG)]
                if eng == "A":
                    nc.scalar.activation(ets[ktp][:], ps2[:], Act.Exp,
                                         scale=scale, bias=shift_sb[:])
                else:
                    ENG[eng].tensor_scalar(ets[ktp][:].bitcast(u8), ps2[:],
                                           s1_schr, s2_schr,
                                           op0=Alu.mult, op1=Alu.add)

            # prologue: q slab 0, then per 1024-token slab: k-proj followed by
            # its 4 score pairs for q-chunk 0, v tiles, remaining q slab
            # scores first within each step: PE is in-order, so a v matmul
            # stalled on its psum slot must not block the score matmuls that
            # feed the (bottleneck) exp engines
            for sc in range(2 * NSLAB):
                emit_kkproj(sc)
                emit_score_pair(0, 2 * sc)
                emit_score_pair(0, 2 * sc + 1)
                emit_v(2 * sc)
                emit_v(2 * sc + 1)

            oe_i = [0]

            def emit_final(rr, last=False):
                osb = outp.tile([P, 2, C], f32, name=f"osb_{rr}", tag="osb", bufs=3)
                ps = psum.tile([P, 2, C], f32, name=f"ops_{rr}", tag="sps", bufs=3)
                for mt in range(2):
                    msl = slice(mt * P, (mt + 1) * P)
                    nc.tensor.matmul(ps[:, mt, :], ones_r[:], bo_rep[:],
                                     start=True, stop=False)
                    nc.tensor.matmul(ps[:, mt, :], ident_sb[:],
                                     xh[:, mt, rr * C:(rr + 1) * C],
                                     start=False, stop=False)
                    nc.tensor.matmul(ps[:, mt, :], h2[2 * rr][:, msl], w_sb["o", 0],
                                     start=False, stop=False)
                    nc.tensor.matmul(ps[:, mt, :], h2[2 * rr + 1][:, msl], w_sb["o", 1],
                                     start=False, stop=True)
                eng = OE_ENG[oe_i[0] % len(OE_ENG)]
                oe_i[0] += 1
                copy8(eng, osb[:], ps[:])
                # one merged DMA for both 128-row blocks of this 256-token column set
                dq = nc.sync
                dq.dma_start(
                    bass.AP(tensor=out_d, offset=rr * C,
                            ap=[[NL, P], [P * NL, 2], [1, C]]),
                    osb[:],
                )

            h2 = []
            h2_i = [0]
            for qc in range(QCH):
                ets = et_chunks[qc]
                for half in range(2):
                    hpss = [
                        psum.tile([P, CP], f32, name=f"hps_{qc}_{half}_{j}",
                                  tag="acc", bufs=2)
                        for j in range(2)
                    ]
                    for ktp in range(KT // 2):
                        # score emission BEFORE this step's h2 matmuls: the
                        # h2 matmuls wait on exp(qc, ktp), and PE is in-order
                        g = half * (KT // 2) + ktp
                        if qc + 1 < QCH and g % 2 == 0:
                            emit_score_pair(qc + 1, g // 2)
                        for j in range(2):
                            qt = 2 * half + j
                            nc.tensor.matmul(hpss[j][:],
                                             ets[ktp][:, :, qt * P:(qt + 1) * P],
                                             v_sb[:, ktp, :, :],
                                             start=(ktp == 0), stop=(ktp == KT // 2 - 1),
                                             perf_mode=mybir.MatmulPerfMode.DoubleRow)
                    for j in range(2):
                        qt = 2 * half + j
                        rec = small.tile([P, 1], f32, name=f"rec_{qc}_{qt}", tag="rec", bufs=4)
                        nc.vector.reciprocal(rec[:], hpss[j][:, C:C + 1])
                        h2t = big.tile([P, C], f32r, name=f"h2_{qc}_{qt}", tag="h2", bufs=6)
                        eng = H2_ENG[h2_i[0] % len(H2_ENG)]
                        h2_i[0] += 1
                        if eng == "A":
                            nc.scalar.mul(h2t[:], hpss[j][:, 0:C], rec[:])
                        else:
                            ENG[eng].tensor_scalar_mul(h2t[:], hpss[j][:, 0:C], rec[:])
                        h2.append(h2t)
                    # final projection for the 256-token block this half completed
                    emit_final(2 * qc + half,
                               last=(qc == QCH - 1 and half == 1))

    nc.compile()
    return nc


def _get_nc():
    if "nc" not in _CACHE:
        _CACHE["nc"] = _build_nc()
    return _CACHE["nc"]


def _make_in_maps(x, gn_w, gn_b, wq, bq, wk, bk, wv, bv, wo, bo):
    x = np.ascontiguousarray(np.asarray(x, dtype=np.float32)).reshape(B, C, N)
    pairm = np.zeros((P, P), dtype=np.float32)
    idx = np.arange(P)
    pairm[idx[:, None] // 2 == idx[None, :] // 2] = 0.5
    wqf = np.asarray(wq, np.float64)
    wkf = np.asarray(wk, np.float64)
    common = {
        "wqk": np.ascontiguousarray((wkf.T @ wqf).astype(np.float32)),
        "wvt": np.ascontiguousarray(np.asarray(wv, np.float32).T),
        "wot": np.ascontiguousarray(np.asarray(wo, np.float32).T),
        "bv": np.asarray(bv, np.float32),
        "bo": np.asarray(bo, np.float32),
        "gnw": np.asarray(gn_w, np.float32),
        "gnb": np.asarray(gn_b, np.float32),
        "pairm": pairm,
        "ident": np.eye(P, dtype=np.float32),
    }
    in_maps = []
    for core in range(NCORES):
        b, half = divmod(core, 2)
        xs = np.roll(x[b], -NL * half, axis=1) if half else x[b]
        in_maps.append({**common, "x": np.ascontiguousarray(xs)})
    return in_maps


def kernel(x, gn_w, gn_b, wq, bq, wk, bk, wv, bv, wo, bo):
    from concourse.bass_utils import run_bass_kernel_spmd

    nc = _get_nc()
    in_maps = _make_in_maps(x, gn_w, gn_b, wq, bq, wk, bk, wv, bv, wo, bo)
    res = run_bass_kernel_spmd(nc, in_maps, core_ids=list(range(NCORES)))
    _CACHE["last_result"] = res

    out = np.empty((B, C, N), dtype=np.float32)
    for core in range(NCORES):
        b, half = divmod(core, 2)
        out[b][:, NL * half:NL * (half + 1)] = res.results[core]["out"]
    return out.reshape(B, C, HH, WW)
